# revision 12
# baseline (speedup 1.0000x reference)
"""Dequantized mixed-sign int8 GEMM on 8 trn2 NeuronCores.

out = ((x - X_ZP) * X_SCALE) @ ((y - Y_ZP) * Y_SCALE)   [4096 x 4096 x 4096]

Strategy: the shifted operands (x+66, y-160) are integers with magnitude
<= 256, exactly representable in bf16 -> run the GEMM as bf16 x bf16 with
fp32 PSUM accumulation (bf16 tensor-engine peak), folding the zero-point
shifts into the on-chip fp32->bf16 conversion and the scale product into
the PSUM->SBUF copy.

Sharding: 4-way over M x 2-way over N (core (mi, nj), mi in 0..3,
nj in 0..1).  Each core gets x[mi].T ([K, 1024] fp32, transposed on host
so K lands on partitions for the stationary operand) and y[:, nj]
([K, 2048] fp32), and produces a [1024, 2048] fp32 output block.
x.T is converted once into SBUF-resident bf16 tiles; y streams through
in 512-wide blocks, double-buffered.
"""

import sys

if "/opt/trn_rl_repo" not in sys.path:
    sys.path.insert(0, "/opt/trn_rl_repo")

import numpy as np

X_SCALE, X_ZP = 0.03, -66.0
Y_SCALE, Y_ZP = 0.025, 160.0
OUT_SCALE = float(np.float32(X_SCALE) * np.float32(Y_SCALE))

M = K = N = 4096
MI, NJ = 4, 2  # core grid: M split x N split
M_SH, N_SH = M // MI, N // NJ  # 1024, 2048 per core
N_CORES = MI * NJ
NBW = 512  # n-block width (one PSUM bank of fp32)


def build(m_sh=M_SH, n_sh=N_SH, k=K, nbw=NBW):
    """Build + compile the per-core Bass program (SPMD: same NEFF on all cores)."""
    from concourse import bacc, mybir, tile

    f32, bf16 = mybir.dt.float32, mybir.dt.bfloat16
    kp = k // 128  # K tiles of 128
    mo_n = m_sh // 128  # M tiles of 128
    nb_n = n_sh // nbw  # N blocks

    nc = bacc.Bacc("TRN2", target_bir_lowering=False, debug=False)
    xt_d = nc.dram_tensor("xt", (k, m_sh), f32, kind="ExternalInput")
    y_d = nc.dram_tensor("y", (k, n_sh), f32, kind="ExternalInput")
    o_d = nc.dram_tensor("o", (m_sh, n_sh), f32, kind="ExternalOutput")

    with tile.TileContext(nc) as tc:
        with (
            tc.tile_pool(name="xstage", bufs=3) as xstage,
            tc.tile_pool(name="ystage", bufs=8) as ystage,
            tc.tile_pool(name="xbf", bufs=1) as xbfp,
            tc.tile_pool(name="ybf", bufs=2) as ybfp,
            tc.tile_pool(name="opool", bufs=4) as opool,
            tc.tile_pool(name="psum", bufs=1, space="PSUM") as psum,
        ):
            def load_y(nb, ko):
                ys = ystage.tile([128, nbw], f32, tag="ys")
                nc.sync.dma_start(
                    ys[:],
                    y_d.ap()[128 * ko : 128 * (ko + 1), nb * nbw : (nb + 1) * nbw],
                )
                yb = ybfp.tile([128, nbw], bf16, tag=f"y{ko}")
                nc.vector.tensor_scalar_add(yb[:], ys[:], -Y_ZP)
                return yb

            # K-tile-interleaved emission: the DMA stream delivers, per K
            # tile, first the y block-0 slice then the x.T slice, so the
            # PE (in-order) can start accumulating as data arrives instead
            # of waiting for the whole resident x to land.
            xbf = []
            ybs0 = []
            for ko in range(kp):
                ybs0.append(load_y(0, ko))
                xs = xstage.tile([128, m_sh], f32, tag="xs")
                nc.sync.dma_start(xs[:], xt_d.ap()[128 * ko : 128 * (ko + 1), :])
                xb = xbfp.tile([128, m_sh], bf16, tag=f"x{ko}")
                # ACT engine: out = Copy(in * 1 + 66); keeps the x-shift off
                # the DVE (which handles the y stream) and off GpSimd (slow).
                nc.scalar.activation(
                    xb[:], xs[:], mybir.ActivationFunctionType.Copy, bias=-X_ZP
                )
                xbf.append(xb)

            for nb in range(nb_n):
                ybs = ybs0 if nb == 0 else [load_y(nb, ko) for ko in range(kp)]

                # All mo_n output chains progress together (one PSUM bank
                # each), interleaved per K step, so every arriving K tile
                # immediately unlocks mo_n matmuls for the in-order PE.
                pss = [
                    psum.tile([128, nbw], f32, tag=f"ps{mo}", name=f"ps{mo}")
                    for mo in range(mo_n)
                ]
                for ko in range(kp):
                    for mo in range(mo_n):
                        nc.tensor.matmul(
                            pss[mo][:],
                            xbf[ko][:, 128 * mo : 128 * (mo + 1)],
                            ybs[ko][:],
                            start=(ko == 0),
                            stop=(ko == kp - 1),
                        )
                for mo in range(mo_n):
                    ot = opool.tile([128, nbw], f32, tag="ot")
                    nc.scalar.activation(
                        ot[:], pss[mo][:], mybir.ActivationFunctionType.Copy,
                        scale=OUT_SCALE,
                    )
                    nc.sync.dma_start(
                        o_d.ap()[128 * mo : 128 * (mo + 1), nb * nbw : (nb + 1) * nbw],
                        ot[:],
                    )

    nc.compile()
    return nc


def build_bf16(m_sh=M_SH, n_sh=N_SH, k=K, nbw=NBW):
    """bf16-input variant: shards arrive as raw bf16 (exact for the
    integer-valued quantized data), pre-tiled on the host so every DMA is
    one fully-contiguous block.  Zero-point shifts happen in-place on chip
    (y on DVE, x on ACT); the scale lands in the PSUM->SBUF copy, which is
    split across DVE/ACT by mo parity so the 8 PSUM banks drain on two
    engines at block boundaries."""
    from concourse import bacc, mybir, tile

    f32, bf16 = mybir.dt.float32, mybir.dt.bfloat16
    kp = k // 128
    mo_n = m_sh // 128
    nb_n = n_sh // nbw
    xg_n = kp // 2  # x groups: [128, 2, m_sh] (two K tiles per load)
    yg_n = kp // 4  # y groups: [128, 4, nbw] (four K tiles per load)

    nc = bacc.Bacc("TRN2", target_bir_lowering=False, debug=False)
    xt_d = nc.dram_tensor("xt", (xg_n, 128, 2, m_sh), bf16, kind="ExternalInput")
    y_d = nc.dram_tensor("y", (nb_n, yg_n, 128, 4, nbw), bf16, kind="ExternalInput")
    o_d = nc.dram_tensor("o", (mo_n, nb_n, 128, nbw), f32, kind="ExternalOutput")

    with tile.TileContext(nc) as tc:
        with (
            tc.tile_pool(name="xbf", bufs=1) as xbfp,
            tc.tile_pool(name="ybf", bufs=3) as ybfp,
            tc.tile_pool(name="opool", bufs=4) as opool,
            tc.tile_pool(name="psum", bufs=1, space="PSUM") as psum,
        ):
            xg = [None] * xg_n

            def load_x(g):
                xb = xbfp.tile([128, 2, m_sh], bf16, tag=f"x{g}", name=f"x{g}")
                nc.sync.dma_start(xb[:], xt_d.ap()[g])
                # alternate the shift between ACT and DVE so neither engine
                # paces the startup stream alone
                if g % 2 == 0:
                    nc.scalar.activation(
                        xb[:], xb[:], mybir.ActivationFunctionType.Copy, bias=-X_ZP
                    )
                else:
                    nc.vector.tensor_scalar_add(xb[:], xb[:], -X_ZP)
                xg[g] = xb

            def load_y(nb):
                tiles = []
                for g in range(yg_n):
                    yb = ybfp.tile(
                        [128, 4, nbw], bf16, tag=f"y{g}", name=f"y{nb}_{g}"
                    )
                    nc.sync.dma_start(yb[:], y_d.ap()[nb, g])
                    nc.vector.tensor_scalar_add(yb[:], yb[:], -Y_ZP)
                    tiles.append(yb)
                return tiles

            def x_slice(ko, mo):  # lhsT [128, 128]
                g, j = divmod(ko, 2)
                return xg[g][:, j, 128 * mo : 128 * (mo + 1)]

            def y_slice(ybs, ko):  # rhs [128, nbw]
                g, j = divmod(ko, 4)
                return ybs[g][:, j, :]

            # Warm-up: the PE clock is HAM-throttled to 1.2 GHz until it has
            # been busy ~3.4us.  While the first input tiles stream in, run
            # dummy matmuls on a zeroed tile into the last PSUM bank (its
            # first real chain starts with start=True, which resets it) so
            # the real chains begin at full 2.4 GHz.
            warm = xbfp.tile([128, 512], bf16, tag="warm", name="warm")
            nc.gpsimd.memset(warm[:], 0.0)
            ps_warm = psum.tile(
                [128, nbw], f32, tag=f"ps{mo_n - 1}", name="ps_warm"
            )
            for _ in range(14):
                nc.tensor.matmul(
                    ps_warm[:], warm[:, :128], warm[:], start=True, stop=True
                )

            # Startup stream: interleave y block 0 with x so the in-order PE
            # can begin accumulating as soon as the first K tiles land.
            ybs0 = []
            for g in range(xg_n):
                if g % 2 == 0 and len(ybs0) < yg_n:
                    gg = g // 2
                    yb = ybfp.tile([128, 4, nbw], bf16, tag=f"y{gg}", name=f"y0_{gg}")
                    nc.sync.dma_start(yb[:], y_d.ap()[0, gg])
                    nc.vector.tensor_scalar_add(yb[:], yb[:], -Y_ZP)
                    ybs0.append(yb)
                load_x(g)

            ybs_next = load_y(1) if nb_n > 1 else None
            ybs = ybs0
            for nb in range(nb_n):
                pss = [
                    psum.tile([128, nbw], f32, tag=f"ps{mo}", name=f"ps{mo}")
                    for mo in range(mo_n)
                ]

                def copy_out(mo):
                    ot = opool.tile([128, nbw], f32, tag="ot", name="ot")
                    if mo % 2 == 0:
                        nc.vector.tensor_scalar_mul(ot[:], pss[mo][:], OUT_SCALE)
                    else:
                        nc.scalar.activation(
                            ot[:], pss[mo][:], mybir.ActivationFunctionType.Copy,
                            scale=OUT_SCALE,
                        )
                    nc.sync.dma_start(o_d.ap()[mo, nb], ot[:])

                if nb == 0:
                    # block 0 is paced by the input stream: interleave all
                    # chains per K step so every arriving K-tile group
                    # unlocks work for the in-order PE
                    for ko in range(kp):
                        for mo in range(mo_n):
                            nc.tensor.matmul(
                                pss[mo][:],
                                x_slice(ko, mo),
                                y_slice(ybs, ko),
                                start=(ko == 0),
                                stop=(ko == kp - 1),
                            )
                    for mo in range(mo_n):
                        copy_out(mo)
                else:
                    # data resident: run chains to completion one at a time
                    # so completions (and PSUM copies) stagger through the
                    # block instead of bursting at its end
                    for mo in range(mo_n):
                        for ko in range(kp):
                            nc.tensor.matmul(
                                pss[mo][:],
                                x_slice(ko, mo),
                                y_slice(ybs, ko),
                                start=(ko == 0),
                                stop=(ko == kp - 1),
                            )
                        copy_out(mo)
                ybs = ybs_next
                ybs_next = load_y(nb + 2) if nb + 2 < nb_n else None

    nc.compile()
    return nc


_nc_cache = {}


def _get_nc(variant="f32"):
    if variant not in _nc_cache:
        _nc_cache[variant] = build() if variant == "f32" else build_bf16()
    return _nc_cache[variant]


def make_in_maps(x: np.ndarray, y: np.ndarray) -> list[dict]:
    x = np.ascontiguousarray(x, dtype=np.float32)
    y = np.ascontiguousarray(y, dtype=np.float32)
    xt_shards = [
        np.ascontiguousarray(x[mi * M_SH : (mi + 1) * M_SH].T) for mi in range(MI)
    ]
    y_shards = [
        np.ascontiguousarray(y[:, nj * N_SH : (nj + 1) * N_SH]) for nj in range(NJ)
    ]
    return [{"xt": xt_shards[i // NJ], "y": y_shards[i % NJ]} for i in range(N_CORES)]


def make_in_maps_bf16(xb: np.ndarray, yb: np.ndarray) -> list[dict]:
    """Pre-tile bf16 shards to match build_bf16's DRAM layouts.

    xt: [K, M_SH] -> [K/256, 128, 2, M_SH]   (contiguous 2-K-tile groups)
    y:  [K, N_SH] -> [NB, K/512, 128, 4, NBW] (contiguous 4-K-tile groups)
    """
    kp = K // 128
    nb_n = N_SH // NBW
    xt_shards = []
    for mi in range(MI):
        xt = xb[mi * M_SH : (mi + 1) * M_SH].T  # [K, M_SH]
        t = xt.reshape(kp // 2, 2, 128, M_SH).transpose(0, 2, 1, 3)
        xt_shards.append(np.ascontiguousarray(t))
    y_shards = []
    for nj in range(NJ):
        ys = yb[:, nj * N_SH : (nj + 1) * N_SH]  # [K, N_SH]
        t = ys.reshape(kp // 4, 4, 128, nb_n, NBW).transpose(3, 0, 2, 1, 4)
        y_shards.append(np.ascontiguousarray(t))
    return [{"xt": xt_shards[i // NJ], "y": y_shards[i % NJ]} for i in range(N_CORES)]


def _cast_bf16_exact(x: np.ndarray, y: np.ndarray):
    """Lossless repack to bf16 when every value survives the cast (true for
    the integer-valued quantized inputs this module targets)."""
    import ml_dtypes

    xb = np.ascontiguousarray(x, dtype=np.float32).astype(ml_dtypes.bfloat16)
    yb = np.ascontiguousarray(y, dtype=np.float32).astype(ml_dtypes.bfloat16)
    if np.array_equal(xb.astype(np.float32), x) and np.array_equal(
        yb.astype(np.float32), y
    ):
        return xb, yb
    return None


def kernel(x: np.ndarray, y: np.ndarray) -> np.ndarray:
    from concourse import bass_utils

    casted = _cast_bf16_exact(x, y)
    if casted is not None:
        nc = _get_nc("bf16")
        in_maps = make_in_maps_bf16(*casted)
    else:  # rare fallback: data not exactly representable in bf16
        nc = _get_nc("f32")
        in_maps = make_in_maps(x, y)

    res = bass_utils.run_bass_kernel_spmd(nc, in_maps, core_ids=list(range(N_CORES)))

    out = np.empty((M, N), dtype=np.float32)
    for i in range(N_CORES):
        mi, nj = i // NJ, i % NJ
        o = res.results[i]["o"]
        if o.ndim == 4:  # [MO, NB, 128, NBW] pre-tiled layout
            o = o.transpose(0, 2, 1, 3).reshape(M_SH, N_SH)
        out[mi * M_SH : (mi + 1) * M_SH, nj * N_SH : (nj + 1) * N_SH] = o
    return out


# revision 13
# speedup vs baseline: 1.1808x; 1.1808x over previous
"""Dequantized mixed-sign int8 GEMM on 8 trn2 NeuronCores.

out = ((x - X_ZP) * X_SCALE) @ ((y - Y_ZP) * Y_SCALE)   [4096 x 4096 x 4096]

Strategy: the shifted operands (x+66, y-160) are integers with magnitude
<= 256, exactly representable in bf16 -> run the GEMM as bf16 x bf16 with
fp32 PSUM accumulation (bf16 tensor-engine peak), folding the zero-point
shifts into the on-chip fp32->bf16 conversion and the scale product into
the PSUM->SBUF copy.

Sharding: 4-way over M x 2-way over N (core (mi, nj), mi in 0..3,
nj in 0..1).  Each core gets x[mi].T ([K, 1024] fp32, transposed on host
so K lands on partitions for the stationary operand) and y[:, nj]
([K, 2048] fp32), and produces a [1024, 2048] fp32 output block.
x.T is converted once into SBUF-resident bf16 tiles; y streams through
in 512-wide blocks, double-buffered.
"""

import sys

if "/opt/trn_rl_repo" not in sys.path:
    sys.path.insert(0, "/opt/trn_rl_repo")

import numpy as np

X_SCALE, X_ZP = 0.03, -66.0
Y_SCALE, Y_ZP = 0.025, 160.0
OUT_SCALE = float(np.float32(X_SCALE) * np.float32(Y_SCALE))

M = K = N = 4096
MI, NJ = 4, 2  # core grid: M split x N split
M_SH, N_SH = M // MI, N // NJ  # 1024, 2048 per core
N_CORES = MI * NJ
NBW = 512  # n-block width (one PSUM bank of fp32)


def build(m_sh=M_SH, n_sh=N_SH, k=K, nbw=NBW):
    """Build + compile the per-core Bass program (SPMD: same NEFF on all cores)."""
    from concourse import bacc, mybir, tile

    f32, bf16 = mybir.dt.float32, mybir.dt.bfloat16
    kp = k // 128  # K tiles of 128
    mo_n = m_sh // 128  # M tiles of 128
    nb_n = n_sh // nbw  # N blocks

    nc = bacc.Bacc("TRN2", target_bir_lowering=False, debug=False)
    xt_d = nc.dram_tensor("xt", (k, m_sh), f32, kind="ExternalInput")
    y_d = nc.dram_tensor("y", (k, n_sh), f32, kind="ExternalInput")
    o_d = nc.dram_tensor("o", (m_sh, n_sh), f32, kind="ExternalOutput")

    with tile.TileContext(nc) as tc:
        with (
            tc.tile_pool(name="xstage", bufs=3) as xstage,
            tc.tile_pool(name="ystage", bufs=8) as ystage,
            tc.tile_pool(name="xbf", bufs=1) as xbfp,
            tc.tile_pool(name="ybf", bufs=2) as ybfp,
            tc.tile_pool(name="opool", bufs=4) as opool,
            tc.tile_pool(name="psum", bufs=1, space="PSUM") as psum,
        ):
            def load_y(nb, ko):
                ys = ystage.tile([128, nbw], f32, tag="ys")
                nc.sync.dma_start(
                    ys[:],
                    y_d.ap()[128 * ko : 128 * (ko + 1), nb * nbw : (nb + 1) * nbw],
                )
                yb = ybfp.tile([128, nbw], bf16, tag=f"y{ko}")
                nc.vector.tensor_scalar_add(yb[:], ys[:], -Y_ZP)
                return yb

            # K-tile-interleaved emission: the DMA stream delivers, per K
            # tile, first the y block-0 slice then the x.T slice, so the
            # PE (in-order) can start accumulating as data arrives instead
            # of waiting for the whole resident x to land.
            xbf = []
            ybs0 = []
            for ko in range(kp):
                ybs0.append(load_y(0, ko))
                xs = xstage.tile([128, m_sh], f32, tag="xs")
                nc.sync.dma_start(xs[:], xt_d.ap()[128 * ko : 128 * (ko + 1), :])
                xb = xbfp.tile([128, m_sh], bf16, tag=f"x{ko}")
                # ACT engine: out = Copy(in * 1 + 66); keeps the x-shift off
                # the DVE (which handles the y stream) and off GpSimd (slow).
                nc.scalar.activation(
                    xb[:], xs[:], mybir.ActivationFunctionType.Copy, bias=-X_ZP
                )
                xbf.append(xb)

            for nb in range(nb_n):
                ybs = ybs0 if nb == 0 else [load_y(nb, ko) for ko in range(kp)]

                # All mo_n output chains progress together (one PSUM bank
                # each), interleaved per K step, so every arriving K tile
                # immediately unlocks mo_n matmuls for the in-order PE.
                pss = [
                    psum.tile([128, nbw], f32, tag=f"ps{mo}", name=f"ps{mo}")
                    for mo in range(mo_n)
                ]
                for ko in range(kp):
                    for mo in range(mo_n):
                        nc.tensor.matmul(
                            pss[mo][:],
                            xbf[ko][:, 128 * mo : 128 * (mo + 1)],
                            ybs[ko][:],
                            start=(ko == 0),
                            stop=(ko == kp - 1),
                        )
                for mo in range(mo_n):
                    ot = opool.tile([128, nbw], f32, tag="ot")
                    nc.scalar.activation(
                        ot[:], pss[mo][:], mybir.ActivationFunctionType.Copy,
                        scale=OUT_SCALE,
                    )
                    nc.sync.dma_start(
                        o_d.ap()[128 * mo : 128 * (mo + 1), nb * nbw : (nb + 1) * nbw],
                        ot[:],
                    )

    nc.compile()
    return nc


def build_bf16(m_sh=M_SH, n_sh=N_SH, k=K, nbw=NBW):
    """bf16-input variant: shards arrive as raw bf16 (exact for the
    integer-valued quantized data), pre-tiled on the host so every DMA is
    one fully-contiguous block.  Zero-point shifts happen in-place on chip
    (y on DVE, x on ACT); the scale lands in the PSUM->SBUF copy, which is
    split across DVE/ACT by mo parity so the 8 PSUM banks drain on two
    engines at block boundaries."""
    from concourse import bacc, mybir, tile

    f32, bf16 = mybir.dt.float32, mybir.dt.bfloat16
    kp = k // 128
    mo_n = m_sh // 128
    nb_n = n_sh // nbw
    xg_n = kp // 2  # x groups: [128, 2, m_sh] (two K tiles per load)
    yg_n = kp // 4  # y groups: [128, 4, nbw] (four K tiles per load)

    nc = bacc.Bacc("TRN2", target_bir_lowering=False, debug=False)
    xt_d = nc.dram_tensor("xt", (xg_n, 128, 2, m_sh), bf16, kind="ExternalInput")
    y_d = nc.dram_tensor("y", (nb_n, yg_n, 128, 4, nbw), bf16, kind="ExternalInput")
    o_d = nc.dram_tensor("o", (mo_n, nb_n, 128, nbw), f32, kind="ExternalOutput")

    with tile.TileContext(nc) as tc:
        with (
            tc.tile_pool(name="xbf", bufs=1) as xbfp,
            tc.tile_pool(name="ybf", bufs=3) as ybfp,
            tc.tile_pool(name="opool", bufs=4) as opool,
            tc.tile_pool(name="psum", bufs=1, space="PSUM") as psum,
        ):
            xg = [None] * xg_n

            def load_x(g):
                xb = xbfp.tile([128, 2, m_sh], bf16, tag=f"x{g}", name=f"x{g}")
                nc.sync.dma_start(xb[:], xt_d.ap()[g])
                # alternate the shift between ACT and DVE so neither engine
                # paces the startup stream alone
                if g % 2 == 0:
                    nc.scalar.activation(
                        xb[:], xb[:], mybir.ActivationFunctionType.Copy, bias=-X_ZP
                    )
                else:
                    nc.vector.tensor_scalar_add(xb[:], xb[:], -X_ZP)
                xg[g] = xb

            def load_y(nb):
                tiles = []
                for g in range(yg_n):
                    yb = ybfp.tile(
                        [128, 4, nbw], bf16, tag=f"y{g}", name=f"y{nb}_{g}"
                    )
                    nc.sync.dma_start(yb[:], y_d.ap()[nb, g])
                    nc.vector.tensor_scalar_add(yb[:], yb[:], -Y_ZP)
                    tiles.append(yb)
                return tiles

            def x_slice(ko, mo):  # lhsT [128, 128]
                g, j = divmod(ko, 2)
                return xg[g][:, j, 128 * mo : 128 * (mo + 1)]

            def y_slice(ybs, ko):  # rhs [128, nbw]
                g, j = divmod(ko, 4)
                return ybs[g][:, j, :]

            # Startup stream: interleave y block 0 with x so the in-order PE
            # can begin accumulating as soon as the first K tiles land.
            ybs0 = []
            for g in range(xg_n):
                if g % 2 == 0 and len(ybs0) < yg_n:
                    gg = g // 2
                    yb = ybfp.tile([128, 4, nbw], bf16, tag=f"y{gg}", name=f"y0_{gg}")
                    nc.sync.dma_start(yb[:], y_d.ap()[0, gg])
                    nc.vector.tensor_scalar_add(yb[:], yb[:], -Y_ZP)
                    ybs0.append(yb)
                load_x(g)

            ybs_next = load_y(1) if nb_n > 1 else None
            ybs = ybs0
            for nb in range(nb_n):
                pss = [
                    psum.tile([128, nbw], f32, tag=f"ps{mo}", name=f"ps{mo}")
                    for mo in range(mo_n)
                ]

                def copy_out(mo):
                    ot = opool.tile([128, nbw], f32, tag="ot", name="ot")
                    if mo % 2 == 0:
                        nc.vector.tensor_scalar_mul(ot[:], pss[mo][:], OUT_SCALE)
                    else:
                        nc.scalar.activation(
                            ot[:], pss[mo][:], mybir.ActivationFunctionType.Copy,
                            scale=OUT_SCALE,
                        )
                    nc.sync.dma_start(o_d.ap()[mo, nb], ot[:])

                if nb == 0:
                    # block 0 is paced by the input stream: interleave all
                    # chains per K step so every arriving K-tile group
                    # unlocks work for the in-order PE
                    for ko in range(kp):
                        for mo in range(mo_n):
                            nc.tensor.matmul(
                                pss[mo][:],
                                x_slice(ko, mo),
                                y_slice(ybs, ko),
                                start=(ko == 0),
                                stop=(ko == kp - 1),
                            )
                    for mo in range(mo_n):
                        copy_out(mo)
                else:
                    # data resident: run chains to completion one at a time
                    # so completions (and PSUM copies) stagger through the
                    # block instead of bursting at its end
                    for mo in range(mo_n):
                        for ko in range(kp):
                            nc.tensor.matmul(
                                pss[mo][:],
                                x_slice(ko, mo),
                                y_slice(ybs, ko),
                                start=(ko == 0),
                                stop=(ko == kp - 1),
                            )
                        copy_out(mo)
                ybs = ybs_next
                ybs_next = load_y(nb + 2) if nb + 2 < nb_n else None

    nc.compile()
    return nc


_nc_cache = {}


def _get_nc(variant="f32"):
    if variant not in _nc_cache:
        _nc_cache[variant] = build() if variant == "f32" else build_bf16()
    return _nc_cache[variant]


def make_in_maps(x: np.ndarray, y: np.ndarray) -> list[dict]:
    x = np.ascontiguousarray(x, dtype=np.float32)
    y = np.ascontiguousarray(y, dtype=np.float32)
    xt_shards = [
        np.ascontiguousarray(x[mi * M_SH : (mi + 1) * M_SH].T) for mi in range(MI)
    ]
    y_shards = [
        np.ascontiguousarray(y[:, nj * N_SH : (nj + 1) * N_SH]) for nj in range(NJ)
    ]
    return [{"xt": xt_shards[i // NJ], "y": y_shards[i % NJ]} for i in range(N_CORES)]


def make_in_maps_bf16(xb: np.ndarray, yb: np.ndarray) -> list[dict]:
    """Pre-tile bf16 shards to match build_bf16's DRAM layouts.

    xt: [K, M_SH] -> [K/256, 128, 2, M_SH]   (contiguous 2-K-tile groups)
    y:  [K, N_SH] -> [NB, K/512, 128, 4, NBW] (contiguous 4-K-tile groups)
    """
    kp = K // 128
    nb_n = N_SH // NBW
    xt_shards = []
    for mi in range(MI):
        xt = xb[mi * M_SH : (mi + 1) * M_SH].T  # [K, M_SH]
        t = xt.reshape(kp // 2, 2, 128, M_SH).transpose(0, 2, 1, 3)
        xt_shards.append(np.ascontiguousarray(t))
    y_shards = []
    for nj in range(NJ):
        ys = yb[:, nj * N_SH : (nj + 1) * N_SH]  # [K, N_SH]
        t = ys.reshape(kp // 4, 4, 128, nb_n, NBW).transpose(3, 0, 2, 1, 4)
        y_shards.append(np.ascontiguousarray(t))
    return [{"xt": xt_shards[i // NJ], "y": y_shards[i % NJ]} for i in range(N_CORES)]


def _cast_bf16_exact(x: np.ndarray, y: np.ndarray):
    """Lossless repack to bf16 when every value survives the cast (true for
    the integer-valued quantized inputs this module targets)."""
    import ml_dtypes

    xb = np.ascontiguousarray(x, dtype=np.float32).astype(ml_dtypes.bfloat16)
    yb = np.ascontiguousarray(y, dtype=np.float32).astype(ml_dtypes.bfloat16)
    if np.array_equal(xb.astype(np.float32), x) and np.array_equal(
        yb.astype(np.float32), y
    ):
        return xb, yb
    return None


def kernel(x: np.ndarray, y: np.ndarray) -> np.ndarray:
    from concourse import bass_utils

    casted = _cast_bf16_exact(x, y)
    if casted is not None:
        nc = _get_nc("bf16")
        in_maps = make_in_maps_bf16(*casted)
    else:  # rare fallback: data not exactly representable in bf16
        nc = _get_nc("f32")
        in_maps = make_in_maps(x, y)

    res = bass_utils.run_bass_kernel_spmd(nc, in_maps, core_ids=list(range(N_CORES)))

    out = np.empty((M, N), dtype=np.float32)
    for i in range(N_CORES):
        mi, nj = i // NJ, i % NJ
        o = res.results[i]["o"]
        if o.ndim == 4:  # [MO, NB, 128, NBW] pre-tiled layout
            o = o.transpose(0, 2, 1, 3).reshape(M_SH, N_SH)
        out[mi * M_SH : (mi + 1) * M_SH, nj * N_SH : (nj + 1) * N_SH] = o
    return out
